# revision 19
# baseline (speedup 1.0000x reference)
"""Single attention head (B=8, S=2048, D=768, H=12) on 8 TRN2 NeuronCores.

Data-parallel over batch (1 element/core). v2 design:
  - x ships as fp16 (3.1MB/core, ~9us DMA floor); host prep is layout only
    (mask permutation packing keys first, d-major chunk layout, fp16 cast).
  - QKV: fp16 weights (optionally hi+lo split accumulated in PE for f32r
    precision). q extracted via DVE (quadrant-aligned), k via DVE, v via ACT.
  - Pass A (row max): 4-way row-tiled f32r matmuls (K=13 at PE rows
    0/32/64/96) split over two 2-bank f32 PSUM tiles; tile X covers keys
    [0, T/2) (ready after key-chunk 1 -> reduces start early), tile Y the
    rest. One DVE reduce per tile, per-chunk final combine.
  - Pass B: same 4-way row tiling with 14 rows (12 q + ones + negmax row)
    over two 2-bank tiles; ACT exp -> p fp16 SBUF.
  - PV: fp16, 2-way col-tiled (M=16 at cols 0/64), f32 PSUM accumulate;
    denominator rides as a ones-column in vaug.
  - Out-stage: batched PE transposes into one PSUM tile per chunk, one
    reciprocal per chunk, gpsimd scalar muls, per-chunk output DMA.
"""

import math
import os

import numpy as np

B, S, D, H = 8, 2048, 768, 12
N_CORES = 8
NCH = 4            # s chunks
SCH = S // NCH     # 512
BIAS_B = -30000.0  # additive mask bias

W_LO = True        # hi/lo weight split for q,k (PE-accumulated)
N_DUMMY = 6        # PE warm-up matmuls


def _build(nc_mod, T_pad):
    bass, mybir, tile, bacc = nc_mod
    f32 = mybir.dt.float32
    f32r = mybir.dt.float32r
    f16 = mybir.dt.float16
    AF = mybir.ActivationFunctionType
    X = mybir.AxisListType.X
    XY = mybir.AxisListType.XY

    NT = (T_pad + 127) // 128
    TR = NT * 128             # key extent rounded to full tiles
    SLAB = TR // 4            # pass-A slab width per row-group

    nc = bacc.Bacc("TRN2", target_bir_lowering=False, debug=False,
                   num_devices=N_CORES)

    x_ext = nc.dram_tensor("x", [128, NCH * 6 * SCH], f16,
                           kind="ExternalInput")
    w_ext = nc.dram_tensor("w", [128, 6 * 192], f16, kind="ExternalInput")
    constB_ext = nc.dram_tensor("constB", [2, TR], f32r,
                                kind="ExternalInput")
    ones_ext = nc.dram_tensor("ones", [1, TR], f32r, kind="ExternalInput")
    ones16_ext = nc.dram_tensor("ones16", [1, TR], f16,
                                kind="ExternalInput")
    out_ext = nc.dram_tensor("out", [128, 256], f32, kind="ExternalOutput")
    DBG = os.environ.get("BASS_DEBUG_DUMP", "0") == "1"
    if DBG:
        dbg_kA = nc.dram_tensor("dbg_kA", [128, TR], f32, kind="ExternalOutput")
        dbg_qc = nc.dram_tensor("dbg_qc", [128, SCH], f32, kind="ExternalOutput")
        dbg_maxc = nc.dram_tensor("dbg_maxc", [128, 16], f32, kind="ExternalOutput")
        dbg_vcomb = nc.dram_tensor("dbg_vcomb", [16, S], f32, kind="ExternalOutput")
        dbg_vaug = nc.dram_tensor("dbg_vaug", [128, NT * 16], f32, kind="ExternalOutput")

    with tile.TileContext(nc) as tc:
        with tc.tile_pool(name="sb", bufs=1) as sb, \
             tc.tile_pool(name="pp", bufs=3) as ppool, \
             tc.tile_pool(name="qv", bufs=1, space="PSUM") as qvp, \
             tc.tile_pool(name="ab", bufs=3, space="PSUM") as abp, \
             tc.tile_pool(name="vp", bufs=1, space="PSUM") as vp:

            xc = [sb.tile([128, 6, SCH], f16, name=f"xc{c}")
                  for c in range(NCH)]
            w = sb.tile([128, 6, 192], f16)
            # kA rows (replicated at 0/32/64/96): 0-11 k, 12 bias, 13 = +1
            kA = sb.tile([128, TR], f32r)
            # qc rows (replicated): 0-11 q, 12 = 1, 13 = -rowmax
            qc = [sb.tile([128, SCH], f32r, name=f"qc{c}")
                  for c in range(NCH)]
            vaugT = sb.tile([16, TR], f16)      # 0-11 v, 12 = 1, 13-15 = 0
            vaug = sb.tile([128, NT, 16], f16)
            identN = sb.tile([128, 128], f32)   # identity (for maxc.T)
            ident16 = sb.tile([16, 16], f16)
            mx2 = sb.tile([128, 16, 2], f32)    # per-half maxes
            maxc = sb.tile([128, 16], f32)      # +rowmax per s-tile
            negmS = sb.tile([1, SCH], f32r)
            vstage = sb.tile([16, SCH], f32)
            vcomb = sb.tile([16, S], f16)
            rec = sb.tile([128, 16], f32)
            outsb = sb.tile([128, 16, 16], f32)

            # ---- input DMAs first: x streams from t=0 on the sync queue ---
            xr = x_ext.ap().rearrange("p (c ko s) -> p c ko s", c=NCH, ko=6)
            for c in range(NCH):
                nc.sync.dma_start(xc[c][:], xr[:, c])
            nc.sync.dma_start(w[:], w_ext.ap().rearrange(
                "p (ko m) -> p ko m", ko=6))
            # ---- constants ----
            nc.gpsimd.memset(vaugT[:, :], 0.0)
            nc.gpsimd.dma_start(vaugT[12:13, :], ones16_ext.ap())
            nc.gpsimd.memset(vaug[:, :, :], 0.0)
            if T_pad < TR:
                nc.gpsimd.memset(kA[:, T_pad:TR].bitcast(f32), 0.0)
            for g in range(4):
                nc.gpsimd.dma_start(kA[32 * g + 12:32 * g + 14, :],
                                    constB_ext.ap())
            from concourse.masks import make_identity
            make_identity(nc, identN[:])
            make_identity(nc, ident16[:])

            # ---- keep the PE HAM-warm during the DMA-bound head ----
            wflat = w[:].rearrange("p ko m -> p (ko m)")
            for i in range(N_DUMMY):
                scr = qvp.tile([128, SCH], f32, tag="qkv", name=f"scr{i}")
                nc.tensor.matmul(scr[0:76, :], w[:, 0, 0:76],
                                 wflat[:, 0:512], start=True, stop=True,
                                 tile_position=(0, 0))

            # ---- QKV projection ----
            # psum rows: 0-11 k, 32-43 q, 64-75 v
            def emit_qkv(c):
                qkv = qvp.tile([128, SCH], f32, tag="qkv", name=f"qkv{c}")
                for ko in range(6):
                    xin = xc[c][:, ko, :]
                    nc.tensor.matmul(qkv[0:76, :], w[:, ko, 0:76], xin,
                                     start=(ko == 0),
                                     stop=(ko == 5 and not W_LO),
                                     tile_position=(0, 0))
                if W_LO:
                    for ko in range(6):
                        xin = xc[c][:, ko, :]
                        nc.tensor.matmul(qkv[0:76, :], w[:, ko, 96:172],
                                         xin, start=False, stop=(ko == 5),
                                         tile_position=(0, 0))
                t0 = c * SCH
                t1 = min((c + 1) * SCH, T_pad)
                if t0 < T_pad:
                    tsl = slice(0, t1 - t0)
                    nc.vector.tensor_copy(kA[0:12, t0:t1], qkv[0:12, tsl])
                    nc.scalar.copy(vaugT[0:12, t0:t1], qkv[64:76, tsl])
                nc.vector.tensor_copy(qc[c][0:12, :], qkv[32:44, :])
                nc.gpsimd.dma_start(qc[c][12:13, :],
                                    ones_ext.ap()[:, 0:SCH])
                for g in range(1, 4):
                    nc.gpsimd.dma_start(qc[c][32 * g:32 * g + 13, :],
                                        qc[c][0:13, :])
                if t0 < T_pad:
                    for g in range(1, 4):
                        nc.gpsimd.dma_start(
                            kA[32 * g:32 * g + 12, t0:t1], kA[0:12, t0:t1])

            # ---- v transposes into PV-stationary layout (batched) ----
            def emit_vt(j0, n):
                ot = abp.tile([128, 4, 16], f16, tag="ab", name=f"vt{j0}")
                for k in range(n):
                    j = j0 + k
                    nc.tensor.transpose(ot[:, k, 0:16],
                                        vaugT[0:16, j * 128:(j + 1) * 128],
                                        ident16[:])
                nc.scalar.copy(vaug[:, j0:j0 + n, :], ot[:, 0:n, :])

            # ---- pass A: 4 row groups over two 2-bank tiles ----
            def emit_A(st, half):
                c, k = st // 4, st % 4
                s0 = k * 128
                pa = abp.tile([128, 2, SCH], f32, tag="ab",
                              name=f"pa{st}_{half}")
                for gg in range(2):
                    g = 2 * half + gg
                    nc.tensor.matmul(
                        pa[:, gg, 0:SLAB],
                        qc[c][32 * g:32 * g + 13, s0:s0 + 128],
                        kA[32 * g:32 * g + 13, g * SLAB:(g + 1) * SLAB],
                        start=True, stop=True, tile_position=(32 * g, 0))
                nc.vector.reduce_max(mx2[:, st, half:half + 1],
                                     pa[:, :, 0:SLAB], axis=XY)

            def emit_maxfin(c):
                nc.vector.reduce_max(maxc[:, 4 * c:4 * c + 4],
                                     mx2[:, 4 * c:4 * c + 4, :], axis=X)

            def emit_negm(c):
                mt = abp.tile([1, SCH], f32, tag="ab", name=f"mt{c}")
                for k in range(4):
                    st = 4 * c + k
                    nc.tensor.transpose(mt[0:1, k * 128:(k + 1) * 128],
                                        maxc[:, st:st + 1], identN[:])
                nc.scalar.copy(negmS[0:1, :], mt[0:1, :])
                for g in range(4):
                    nc.gpsimd.dma_start(
                        qc[c][32 * g + 13:32 * g + 14, :], negmS[0:1, :])

            # ---- pass B + exp + PV ----
            jgroups = []
            j = 0
            while j < NT:
                jgroups.append(list(range(j, min(j + 4, NT))))
                j += 4
            pv_last = {0: max(j for j in range(NT) if j % 2 == 0),
                       1: max((j for j in range(NT) if j % 2 == 1),
                              default=-1)}

            def emit_B(c, gi):
                grp = jgroups[gi]
                p = ppool.tile([128, 4, SCH], f16, tag="p",
                               name=f"p{c}_{gi}")
                for h in range(0, len(grp), 2):
                    sub = grp[h:h + 2]
                    bt = abp.tile([128, 2, SCH], f32, tag="ab",
                                  name=f"bt{c}_{gi}_{h}")
                    for gg, j in enumerate(sub):
                        g = h + gg
                        nc.tensor.matmul(
                            bt[:, gg, :],
                            kA[32 * g:32 * g + 14, j * 128:(j + 1) * 128],
                            qc[c][32 * g:32 * g + 14, :],
                            start=True, stop=True,
                            tile_position=(32 * g, 0))
                    nc.scalar.activation(p[:, h:h + len(sub), :],
                                         bt[:, 0:len(sub), :], AF.Exp)
                return p

            def emit_PV(c, gi, p, vacc):
                for g, j in enumerate(jgroups[gi]):
                    col = 64 * (j % 2)
                    nc.tensor.matmul(
                        vacc[col:col + 16, :], vaug[:, j, 0:16], p[:, g, :],
                        start=(j < 2), stop=(j == pv_last[j % 2]),
                        tile_position=(0, col))

            def emit_drain(c, vacc):
                cs = slice(c * SCH, (c + 1) * SCH)
                nc.scalar.copy(vstage[0:16, :], vacc[64:80, :])
                nc.vector.tensor_add(vcomb[0:16, cs], vacc[0:16, :],
                                     vstage[0:16, :])

            def emit_out(c):
                ot = abp.tile([128, 4, 16], f16, tag="ab", name=f"ot{c}")
                for k in range(4):
                    st = 4 * c + k
                    nc.tensor.transpose(
                        ot[:, k, 0:16],
                        vcomb[0:16, st * 128:(st + 1) * 128], ident16[:])
                nc.scalar.copy(outsb[:, 4 * c:4 * c + 4, :], ot[:, :, :])
                nc.vector.reciprocal(rec[:, 4 * c:4 * c + 4],
                                     outsb[:, 4 * c:4 * c + 4, 12:13])
                for k in range(4):
                    st = 4 * c + k
                    nc.gpsimd.tensor_scalar_mul(
                        outsb[:, st, 0:12], outsb[:, st, 0:12],
                        rec[:, st:st + 1])
                outr = out_ext.ap().rearrange("p (a b) -> p a b", a=16)
                nc.sync.dma_start(outr[:, 4 * c:4 * c + 4, :],
                                  outsb[:, 4 * c:4 * c + 4, :])

            # ---- schedule ----
            emit_qkv(0)
            emit_vt(0, 4)
            emit_qkv(1)
            if NT > 4:
                emit_vt(4, min(4, NT - 4))
            # keys half X = slabs 0-1 (cols < TR/2) ready after chunk 1
            for st in range(0, 8):
                emit_A(st, 0)
            emit_qkv(2)
            if NT > 8:
                emit_vt(8, NT - 8)
            for st in range(0, 4):
                emit_A(st, 1)
            emit_maxfin(0)
            emit_negm(0)
            for st in range(4, 8):
                emit_A(st, 1)
            emit_maxfin(1)
            emit_negm(1)
            emit_qkv(3)
            for st in range(8, 12):
                emit_A(st, 0)
                emit_A(st, 1)
            emit_maxfin(2)
            emit_negm(2)
            for st in range(12, 16):
                emit_A(st, 0)
                emit_A(st, 1)
            emit_maxfin(3)
            emit_negm(3)

            for c in range(NCH):
                vacc = vp.tile([128, SCH], f32, tag="vacc", name=f"vacc{c}")
                ps = []
                for gi in range(len(jgroups)):
                    ps.append(emit_B(c, gi))
                    if gi >= 1:
                        emit_PV(c, gi - 1, ps[gi - 1], vacc)
                emit_PV(c, len(jgroups) - 1, ps[-1], vacc)
                emit_drain(c, vacc)
                if c >= 1:
                    emit_out(c - 1)
            emit_out(NCH - 1)

            if DBG:
                dkA = sb.tile([128, TR], f32)
                dqc = sb.tile([128, SCH], f32)
                dvg = sb.tile([128, NT, 16], f32)
                nc.vector.tensor_copy(dkA[:], kA[:].bitcast(f32))
                nc.vector.tensor_copy(dqc[:], qc[0][:].bitcast(f32))
                nc.vector.tensor_copy(dvg[:], vaug[:])
                nc.sync.dma_start(dbg_kA.ap(), dkA[:])
                nc.sync.dma_start(dbg_qc.ap(), dqc[:])
                nc.sync.dma_start(dbg_maxc.ap(), maxc[:])
                dvc = sb.tile([16, S], f32)
                nc.vector.tensor_copy(dvc[:], vcomb[:])
                nc.sync.dma_start(dbg_vcomb.ap(), dvc[:])
                nc.sync.dma_start(dbg_vaug.ap().rearrange(
                    "p (j n) -> p j n", j=NT), dvg[:])

    nc.compile()
    return nc


def kernel(x, mask, key_weight, query_weight, value_weight):
    import concourse.bass as bass
    import concourse.mybir as mybir
    import concourse.tile as tile
    from concourse import bacc, bass_utils

    x = np.asarray(x, dtype=np.float32)
    mask = np.asarray(mask)
    wk = np.asarray(key_weight, dtype=np.float32)
    wq = np.asarray(query_weight, dtype=np.float32)
    wv = np.asarray(value_weight, dtype=np.float32)

    # weight packing: cols 0-11 k, 32-43 q, 64-75 v; lo residual at +96
    w2 = np.zeros((D, 192), dtype=np.float32)
    w2[:, 0:12] = wk
    w2[:, 32:44] = wq / math.sqrt(H)
    w2[:, 64:76] = wv
    w_hi = w2.astype(np.float16).astype(np.float32)
    w_lo = w2 - w_hi
    wpack = np.zeros((D, 192), dtype=np.float16)
    wpack[:, 0:76] = w_hi[:, 0:76].astype(np.float16)
    if W_LO:
        wpack[:, 96:108] = w_lo[:, 0:12].astype(np.float16)
        wpack[:, 128:140] = w_lo[:, 32:44].astype(np.float16)
        wpack[:, 160:172] = w_lo[:, 64:76].astype(np.float16)
    w_dev = np.ascontiguousarray(
        wpack.reshape(6, 128, 192).transpose(1, 0, 2)).reshape(128, 6 * 192)

    perms, nbs = [], []
    for b in range(B):
        m = mask[b, 0].astype(np.int64)
        perm = np.argsort(1 - m, kind="stable")
        perms.append(perm)
        nbs.append(int(m.sum()))
    T_pad = max(128, int(np.ceil(max(max(nbs), 1) / 32.0)) * 32)
    T_pad = min(T_pad, S)
    TR = ((T_pad + 127) // 128) * 128

    in_maps = []
    for b in range(B):
        xp = x[b][perms[b]].astype(np.float16)     # [S, D]
        xp = xp.reshape(NCH, SCH, 6, 128)          # [c, s, ko, p]
        x_dev = np.ascontiguousarray(
            xp.transpose(3, 0, 2, 1)).reshape(128, NCH * 6 * SCH)
        constB = np.zeros((2, TR), dtype=np.float32)
        constB[0, nbs[b]:] = BIAS_B
        constB[1, :] = -1.0
        in_maps.append({"x": x_dev, "w": w_dev, "constB": constB,
                        "ones": np.ones((1, TR), dtype=np.float32),
                        "ones16": np.ones((1, TR), dtype=np.float16)})

    import time as _time
    _t0 = _time.time()
    print(f"[kernel] building graph, T_pad={T_pad} TR={TR}", flush=True)
    nc = _build((bass, mybir, tile, bacc), T_pad)
    print(f"[kernel] graph+bacc compile done in {_time.time() - _t0:.1f}s",
          flush=True)

    trace = os.environ.get("BASS_KERNEL_TRACE", "0") == "1"
    if trace:
        import sys
        import types
        from trn_agent_boot.trn_boot import _ntff_profile_via_ctypes
        hook = _ntff_profile_via_ctypes("/opt/axon/libaxon_pjrt.so")
        m = types.ModuleType("antenv.axon_hooks")
        m.get_axon_ntff_profile_hook = lambda: hook
        sys.modules["antenv.axon_hooks"] = m
        bass_utils.upload_artifacts = lambda tmpdir: "local://" + tmpdir

    res = bass_utils.run_bass_kernel_spmd(
        nc, in_maps, core_ids=list(range(N_CORES)), trace=trace)
    if trace:
        print(f"HW exec time: {res.exec_time_ns} ns", flush=True)
        global _last_res
        _last_res = res

    out = np.empty((B, S, H), dtype=np.float32)
    for b in range(B):
        o = res.results[b]["out"].reshape(128, 16, 16)[:, :, :H]
        out[b, perms[b], :] = o.transpose(1, 0, 2).reshape(S, H)
    return out


# revision 23
# speedup vs baseline: 1.0304x; 1.0304x over previous
"""Single attention head (B=8, S=2048, D=768, H=12) on 8 TRN2 NeuronCores.

Data-parallel over batch (1 element/core). v2 design:
  - x ships as fp16 (3.1MB/core, ~9us DMA floor); host prep is layout only
    (mask permutation packing keys first, d-major chunk layout, fp16 cast).
  - QKV: fp16 weights (optionally hi+lo split accumulated in PE for f32r
    precision). q extracted via DVE (quadrant-aligned), k via DVE, v via ACT.
  - Pass A (row max): 4-way row-tiled f32r matmuls (K=13 at PE rows
    0/32/64/96) split over two 2-bank f32 PSUM tiles; tile X covers keys
    [0, T/2) (ready after key-chunk 1 -> reduces start early), tile Y the
    rest. One DVE reduce per tile, per-chunk final combine.
  - Pass B: same 4-way row tiling with 14 rows (12 q + ones + negmax row)
    over two 2-bank tiles; ACT exp -> p fp16 SBUF.
  - PV: fp16, 2-way col-tiled (M=16 at cols 0/64), f32 PSUM accumulate;
    denominator rides as a ones-column in vaug.
  - Out-stage: batched PE transposes into one PSUM tile per chunk, one
    reciprocal per chunk, gpsimd scalar muls, per-chunk output DMA.
"""

import math
import os

import numpy as np

B, S, D, H = 8, 2048, 768, 12
N_CORES = 8
NCH = 4            # s chunks
SCH = S // NCH     # 512
BIAS_B = -30000.0  # additive mask bias

W_LO = True        # hi/lo weight split for q,k (PE-accumulated)
N_DUMMY = 6        # PE warm-up matmuls


def _build(nc_mod, T_pad):
    bass, mybir, tile, bacc = nc_mod
    f32 = mybir.dt.float32
    f32r = mybir.dt.float32r
    f16 = mybir.dt.float16
    AF = mybir.ActivationFunctionType
    X = mybir.AxisListType.X
    XY = mybir.AxisListType.XY

    NT = (T_pad + 127) // 128
    TR = NT * 128             # key extent rounded to full tiles
    SLAB = TR // 4            # pass-A slab width per row-group

    nc = bacc.Bacc("TRN2", target_bir_lowering=False, debug=False,
                   num_devices=N_CORES)

    x_ext = nc.dram_tensor("x", [128, NCH * 6 * SCH], f16,
                           kind="ExternalInput")
    w_ext = nc.dram_tensor("w", [128, 6 * 192], f16, kind="ExternalInput")
    constB_ext = nc.dram_tensor("constB", [2, TR], f32r,
                                kind="ExternalInput")
    ones_ext = nc.dram_tensor("ones", [1, TR], f32r, kind="ExternalInput")
    ones16_ext = nc.dram_tensor("ones16", [1, TR], f16,
                                kind="ExternalInput")
    out_ext = nc.dram_tensor("out", [128, 256], f32, kind="ExternalOutput")
    DBG = os.environ.get("BASS_DEBUG_DUMP", "0") == "1"
    if DBG:
        dbg_kA = nc.dram_tensor("dbg_kA", [128, TR], f32, kind="ExternalOutput")
        dbg_qc = nc.dram_tensor("dbg_qc", [128, SCH], f32, kind="ExternalOutput")
        dbg_maxc = nc.dram_tensor("dbg_maxc", [128, 16], f32, kind="ExternalOutput")
        dbg_vcomb = nc.dram_tensor("dbg_vcomb", [16, S], f32, kind="ExternalOutput")
        dbg_vaug = nc.dram_tensor("dbg_vaug", [128, NT * 16], f32, kind="ExternalOutput")

    with tile.TileContext(nc) as tc:
        with tc.tile_pool(name="sb", bufs=1) as sb, \
             tc.tile_pool(name="pp", bufs=3) as ppool, \
             tc.tile_pool(name="qv", bufs=2, space="PSUM") as qvp, \
             tc.tile_pool(name="ab", bufs=2, space="PSUM") as abp, \
             tc.tile_pool(name="vp", bufs=2, space="PSUM") as vp:

            xc = [sb.tile([128, 6, SCH], f16, name=f"xc{c}")
                  for c in range(NCH)]
            w = sb.tile([128, 6, 192], f16)
            # kA rows (replicated at 0/32/64/96): 0-11 k, 12 bias, 13 = +1
            kA = sb.tile([128, TR], f32r)
            # qc rows (replicated): 0-11 q, 12 = 1, 13 = -rowmax
            qc = [sb.tile([128, SCH], f32r, name=f"qc{c}")
                  for c in range(NCH)]
            vaugT = sb.tile([16, TR], f16)      # 0-11 v, 12 = 1, 13-15 = 0
            vaug = sb.tile([128, NT, 16], f16)
            identN = sb.tile([128, 128], f32)   # identity (for maxc.T)
            ident16 = sb.tile([16, 16], f16)
            mx2 = sb.tile([128, 16, 2], f32)    # per-half maxes
            maxc = sb.tile([128, 16], f32)      # +rowmax per s-tile
            negmS = sb.tile([1, SCH], f32r)
            vstage = sb.tile([16, SCH], f32)
            vcomb = sb.tile([16, S], f16)
            rec = sb.tile([128, 16], f32)
            outsb = sb.tile([128, 16, 16], f32)

            # ---- input DMAs first: x streams from t=0 on the sync queue ---
            xr = x_ext.ap().rearrange("p (c ko s) -> p c ko s", c=NCH, ko=6)
            for c in range(NCH):
                nc.sync.dma_start(xc[c][:], xr[:, c])
            nc.sync.dma_start(w[:], w_ext.ap().rearrange(
                "p (ko m) -> p ko m", ko=6))
            # ---- constants ----
            nc.gpsimd.memset(vaugT[:, :], 0.0)
            nc.scalar.dma_start(vaugT[12:13, :], ones16_ext.ap())
            nc.gpsimd.memset(vaug[:, :, :], 0.0)
            if T_pad < TR:
                nc.gpsimd.memset(kA[:, T_pad:TR].bitcast(f32), 0.0)
            for g in range(4):
                nc.scalar.dma_start(kA[32 * g + 12:32 * g + 14, :],
                                    constB_ext.ap())
            from concourse.masks import make_identity
            make_identity(nc, identN[:])
            make_identity(nc, ident16[:])

            # ---- keep the PE HAM-warm during the DMA-bound head ----
            wflat = w[:].rearrange("p ko m -> p (ko m)")
            for i in range(N_DUMMY):
                scr = qvp.tile([128, SCH], f32, tag="qkv", name=f"scr{i}")
                nc.tensor.matmul(scr[0:76, :], w[:, 0, 0:76],
                                 wflat[:, 0:512], start=True, stop=True,
                                 tile_position=(0, 0))

            # ---- QKV projection ----
            # psum rows: 0-11 k, 32-43 q, 64-75 v
            def emit_qkv(c):
                qkv = qvp.tile([128, SCH], f32, tag="qkv", name=f"qkv{c}")
                for ko in range(6):
                    xin = xc[c][:, ko, :]
                    nc.tensor.matmul(qkv[0:76, :], w[:, ko, 0:76], xin,
                                     start=(ko == 0),
                                     stop=(ko == 5 and not W_LO),
                                     tile_position=(0, 0))
                if W_LO:
                    for ko in range(6):
                        xin = xc[c][:, ko, :]
                        nc.tensor.matmul(qkv[0:76, :], w[:, ko, 96:172],
                                         xin, start=False, stop=(ko == 5),
                                         tile_position=(0, 0))
                t0 = c * SCH
                t1 = min((c + 1) * SCH, T_pad)
                if t0 < T_pad:
                    tsl = slice(0, t1 - t0)
                    nc.vector.tensor_copy(kA[0:12, t0:t1], qkv[0:12, tsl])
                    nc.scalar.copy(vaugT[0:12, t0:t1], qkv[64:76, tsl])
                nc.vector.tensor_copy(qc[c][0:12, :], qkv[32:44, :])
                nc.sync.dma_start(qc[c][12:13, :], ones_ext.ap()[:, 0:SCH])
                for g in range(1, 4):
                    nc.gpsimd.dma_start(qc[c][32 * g:32 * g + 13, :],
                                        qc[c][0:13, :])
                if t0 < T_pad:
                    for g in range(1, 4):
                        nc.gpsimd.dma_start(
                            kA[32 * g:32 * g + 12, t0:t1], kA[0:12, t0:t1])

            # ---- v transposes into PV-stationary layout (batched) ----
            def emit_vt(j0, n):
                ot = abp.tile([128, 4, 16], f16, tag="ab", name=f"vt{j0}")
                for k in range(n):
                    j = j0 + k
                    nc.tensor.transpose(ot[:, k, 0:16],
                                        vaugT[0:16, j * 128:(j + 1) * 128],
                                        ident16[:])
                nc.scalar.copy(vaug[:, j0:j0 + n, :], ot[:, 0:n, :])

            # ---- pass A: 4 row groups over two 2-bank tiles ----
            def emit_A(st, half):
                c, k = st // 4, st % 4
                s0 = k * 128
                pa = abp.tile([128, 2, SCH], f32, tag="ab",
                              name=f"pa{st}_{half}")
                for gg in range(2):
                    g = 2 * half + gg
                    nc.tensor.matmul(
                        pa[:, gg, 0:SLAB],
                        qc[c][32 * g:32 * g + 13, s0:s0 + 128],
                        kA[32 * g:32 * g + 13, g * SLAB:(g + 1) * SLAB],
                        start=True, stop=True, tile_position=(32 * g, 0))
                nc.vector.reduce_max(mx2[:, st, half:half + 1],
                                     pa[:, :, 0:SLAB], axis=XY)

            def emit_maxfin(c):
                nc.vector.reduce_max(maxc[:, 4 * c:4 * c + 4],
                                     mx2[:, 4 * c:4 * c + 4, :], axis=X)

            def emit_negm(c):
                mt = abp.tile([1, SCH], f32, tag="ab", name=f"mt{c}")
                for k in range(4):
                    st = 4 * c + k
                    nc.tensor.transpose(mt[0:1, k * 128:(k + 1) * 128],
                                        maxc[:, st:st + 1], identN[:])
                nc.scalar.copy(negmS[0:1, :], mt[0:1, :])
                for g in range(4):
                    nc.gpsimd.dma_start(
                        qc[c][32 * g + 13:32 * g + 14, :], negmS[0:1, :])

            # ---- pass B + exp + PV ----
            jgroups = []
            j = 0
            while j < NT:
                jgroups.append(list(range(j, min(j + 4, NT))))
                j += 4
            pv_last = {0: max(j for j in range(NT) if j % 2 == 0),
                       1: max((j for j in range(NT) if j % 2 == 1),
                              default=-1)}

            def emit_B(c, gi):
                grp = jgroups[gi]
                p = ppool.tile([128, 4, SCH], f16, tag="p",
                               name=f"p{c}_{gi}")
                for h in range(0, len(grp), 2):
                    sub = grp[h:h + 2]
                    bt = abp.tile([128, 2, SCH], f32, tag="ab",
                                  name=f"bt{c}_{gi}_{h}")
                    for gg, j in enumerate(sub):
                        g = h + gg
                        nc.tensor.matmul(
                            bt[:, gg, :],
                            kA[32 * g:32 * g + 14, j * 128:(j + 1) * 128],
                            qc[c][32 * g:32 * g + 14, :],
                            start=True, stop=True,
                            tile_position=(32 * g, 0))
                    nc.scalar.activation(p[:, h:h + len(sub), :],
                                         bt[:, 0:len(sub), :], AF.Exp)
                return p

            def emit_PV(c, gi, p, vacc):
                for g, j in enumerate(jgroups[gi]):
                    col = 64 * (j % 2)
                    nc.tensor.matmul(
                        vacc[col:col + 16, :], vaug[:, j, 0:16], p[:, g, :],
                        start=(j < 2), stop=(j == pv_last[j % 2]),
                        tile_position=(0, col))

            def emit_drain(c, vacc):
                cs = slice(c * SCH, (c + 1) * SCH)
                nc.scalar.copy(vstage[0:16, :], vacc[64:80, :])
                nc.vector.tensor_add(vcomb[0:16, cs], vacc[0:16, :],
                                     vstage[0:16, :])

            def emit_out(c):
                ot = abp.tile([128, 4, 16], f16, tag="ab", name=f"ot{c}")
                for k in range(4):
                    st = 4 * c + k
                    nc.tensor.transpose(
                        ot[:, k, 0:16],
                        vcomb[0:16, st * 128:(st + 1) * 128], ident16[:])
                nc.scalar.copy(outsb[:, 4 * c:4 * c + 4, :], ot[:, :, :])
                nc.vector.reciprocal(rec[:, 4 * c:4 * c + 4],
                                     outsb[:, 4 * c:4 * c + 4, 12:13])
                for k in range(4):
                    st = 4 * c + k
                    nc.gpsimd.tensor_scalar_mul(
                        outsb[:, st, 0:12], outsb[:, st, 0:12],
                        rec[:, st:st + 1])
                outr = out_ext.ap().rearrange("p (a b) -> p a b", a=16)
                nc.sync.dma_start(outr[:, 4 * c:4 * c + 4, :],
                                  outsb[:, 4 * c:4 * c + 4, :])

            # ---- schedule ----
            def emit_Ablock(c):
                for st in range(4 * c, 4 * c + 4):
                    emit_A(st, 0)
                    emit_A(st, 1)
                emit_maxfin(c)
                emit_negm(c)

            def emit_Bblock(c):
                vacc = vp.tile([128, SCH], f32, tag="vacc", name=f"vacc{c}")
                ps = []
                for gi in range(len(jgroups)):
                    ps.append(emit_B(c, gi))
                    if gi >= 1:
                        emit_PV(c, gi - 1, ps[gi - 1], vacc)
                emit_PV(c, len(jgroups) - 1, ps[-1], vacc)
                emit_drain(c, vacc)

            for c in range(NCH):
                emit_qkv(c)
            emit_vt(0, 4)
            if NT > 4:
                emit_vt(4, min(4, NT - 4))
            if NT > 8:
                emit_vt(8, NT - 8)
            emit_Ablock(0)
            emit_Ablock(1)
            emit_Bblock(0)
            emit_Ablock(2)
            emit_Bblock(1)
            emit_out(0)
            emit_Ablock(3)
            emit_Bblock(2)
            emit_out(1)
            emit_Bblock(3)
            emit_out(2)
            emit_out(NCH - 1)

            if DBG:
                dkA = sb.tile([128, TR], f32)
                dqc = sb.tile([128, SCH], f32)
                dvg = sb.tile([128, NT, 16], f32)
                nc.vector.tensor_copy(dkA[:], kA[:].bitcast(f32))
                nc.vector.tensor_copy(dqc[:], qc[0][:].bitcast(f32))
                nc.vector.tensor_copy(dvg[:], vaug[:])
                nc.sync.dma_start(dbg_kA.ap(), dkA[:])
                nc.sync.dma_start(dbg_qc.ap(), dqc[:])
                nc.sync.dma_start(dbg_maxc.ap(), maxc[:])
                dvc = sb.tile([16, S], f32)
                nc.vector.tensor_copy(dvc[:], vcomb[:])
                nc.sync.dma_start(dbg_vcomb.ap(), dvc[:])
                nc.sync.dma_start(dbg_vaug.ap().rearrange(
                    "p (j n) -> p j n", j=NT), dvg[:])

    nc.compile()
    return nc


def kernel(x, mask, key_weight, query_weight, value_weight):
    import concourse.bass as bass
    import concourse.mybir as mybir
    import concourse.tile as tile
    from concourse import bacc, bass_utils

    x = np.asarray(x, dtype=np.float32)
    mask = np.asarray(mask)
    wk = np.asarray(key_weight, dtype=np.float32)
    wq = np.asarray(query_weight, dtype=np.float32)
    wv = np.asarray(value_weight, dtype=np.float32)

    # weight packing: cols 0-11 k, 32-43 q, 64-75 v; lo residual at +96
    w2 = np.zeros((D, 192), dtype=np.float32)
    w2[:, 0:12] = wk
    w2[:, 32:44] = wq / math.sqrt(H)
    w2[:, 64:76] = wv
    w_hi = w2.astype(np.float16).astype(np.float32)
    w_lo = w2 - w_hi
    wpack = np.zeros((D, 192), dtype=np.float16)
    wpack[:, 0:76] = w_hi[:, 0:76].astype(np.float16)
    if W_LO:
        wpack[:, 96:108] = w_lo[:, 0:12].astype(np.float16)
        wpack[:, 128:140] = w_lo[:, 32:44].astype(np.float16)
        wpack[:, 160:172] = w_lo[:, 64:76].astype(np.float16)
    w_dev = np.ascontiguousarray(
        wpack.reshape(6, 128, 192).transpose(1, 0, 2)).reshape(128, 6 * 192)

    perms, nbs = [], []
    for b in range(B):
        m = mask[b, 0].astype(np.int64)
        perm = np.argsort(1 - m, kind="stable")
        perms.append(perm)
        nbs.append(int(m.sum()))
    T_pad = max(128, int(np.ceil(max(max(nbs), 1) / 32.0)) * 32)
    T_pad = min(T_pad, S)
    TR = ((T_pad + 127) // 128) * 128

    in_maps = []
    for b in range(B):
        xp = x[b][perms[b]].astype(np.float16)     # [S, D]
        xp = xp.reshape(NCH, SCH, 6, 128)          # [c, s, ko, p]
        x_dev = np.ascontiguousarray(
            xp.transpose(3, 0, 2, 1)).reshape(128, NCH * 6 * SCH)
        constB = np.zeros((2, TR), dtype=np.float32)
        constB[0, nbs[b]:] = BIAS_B
        constB[1, :] = -1.0
        in_maps.append({"x": x_dev, "w": w_dev, "constB": constB,
                        "ones": np.ones((1, TR), dtype=np.float32),
                        "ones16": np.ones((1, TR), dtype=np.float16)})

    import time as _time
    _t0 = _time.time()
    print(f"[kernel] building graph, T_pad={T_pad} TR={TR}", flush=True)
    nc = _build((bass, mybir, tile, bacc), T_pad)
    print(f"[kernel] graph+bacc compile done in {_time.time() - _t0:.1f}s",
          flush=True)

    trace = os.environ.get("BASS_KERNEL_TRACE", "0") == "1"
    if trace:
        import sys
        import types
        from trn_agent_boot.trn_boot import _ntff_profile_via_ctypes
        hook = _ntff_profile_via_ctypes("/opt/axon/libaxon_pjrt.so")
        m = types.ModuleType("antenv.axon_hooks")
        m.get_axon_ntff_profile_hook = lambda: hook
        sys.modules["antenv.axon_hooks"] = m
        bass_utils.upload_artifacts = lambda tmpdir: "local://" + tmpdir

    res = bass_utils.run_bass_kernel_spmd(
        nc, in_maps, core_ids=list(range(N_CORES)), trace=trace)
    if trace:
        print(f"HW exec time: {res.exec_time_ns} ns", flush=True)
        global _last_res
        _last_res = res

    out = np.empty((B, S, H), dtype=np.float32)
    for b in range(B):
        o = res.results[b]["out"].reshape(128, 16, 16)[:, :, :H]
        out[b, perms[b], :] = o.transpose(1, 0, 2).reshape(S, H)
    return out
